# revision 10
# baseline (speedup 1.0000x reference)
"""Trainium2 Bass kernel for nn_CustomAttentionLayer (GQA attention + RoPE + o_proj).

Sharding: 8-way over (batch, query-chunk): core c handles batch c//4, query rows
[(c%4)*512, (c%4)*512+512). Each core computes full K/V for its batch (4x
redundant k/v projection, zero collectives), attention for all 16 heads on its
512 query rows, and the output projection for those rows.

Precision/layout design (validated numerically and on HW):
- Projections and o_proj run as fp8-e4m3 "split" matmuls in DoubleRow perf
  mode (256-deep contraction per instruction at 0.5 cyc/row): X ~= X8 + Xr
  with the residual stored unscaled in fp8 (subnormals carry it), weights
  prescaled by 64 on the host so the residual of W8 stays representable.
  Three DR terms (X8@W8 + X8@Wr + Xr@W8) accumulate in ONE fp32 PSUM group:
  0.75x the PE cost of fp16 at ~bf16-grade accuracy. The 64x output scale is
  folded into exp()'s scale on the scores path, a 64.0-valued ones-matmul on
  the rowsum path, and a final /64 on the host for the o_proj output.
- Attention core (scores, exp, attn.V) is fp16: the probs tensor can only be
  written once (one ACT pass over 16.7M scores is the floor), and fp8
  probs/v fail the error budget (quantization noise does not average out in
  a softmax-weighted mean).
- Softmax rowsums: fp16 tree-adds of the exp tiles on DVE (2x mode) + one
  128-contraction ones-matmul per head (the full ones-matmul rowsum of the
  f32r baseline was 54.6us of pure PE time).
- Rope: PSUM evacuated to fp16 by DVE, then 4 fp16 tensor ops at DVE 2x.
"""

import os
import numpy as np
import ml_dtypes

import concourse.bass as bass
import concourse.mybir as mybir
import concourse.tile as tile
from concourse import bacc
from concourse.bass_utils import run_bass_kernel_spmd

B, S, H = 2, 2048, 2048
NH, NKV, HD = 16, 4, 128
SQ = 512                      # query rows per core
NC = 8                        # cores
SCALE = 1.0 / float(np.sqrt(HD))
WS = 64.0                     # weight prescale (folded back out downstream)

f32 = mybir.dt.float32
f16 = mybir.dt.float16
fp8 = mybir.dt.float8e4
DR = mybir.MatmulPerfMode.DoubleRow
FP = mybir.ActivationFunctionType
ALU = mybir.AluOpType
np8 = ml_dtypes.float8_e4m3


def _body(nc, tc, t):
    h8D, hr8D = t["h8"], t["hr8"]
    wq8D, wqr8D = t["wq8"], t["wqr8"]
    wk8D, wkr8D = t["wk8"], t["wkr8"]
    wv8D, wvr8D = t["wv8"], t["wvr8"]
    wo8D, wor8D = t["wo8"], t["wor8"]
    ccD, ssD, outD = t["cc"], t["ss"], t["out"]

    with tc.tile_pool(name="main", bufs=1) as main, tc.tile_pool(
        name="psum", bufs=1, space="PSUM"
    ) as pp:
        cc = main.tile([128, S], f16, tag="cc", bufs=1)
        ss = main.tile([128, S], f16, tag="ss", bufs=1)
        qts = [main.tile([128, SQ], f16, tag="qt", bufs=NH, name=f"qt{i}")
               for i in range(NH)]
        kts = [main.tile([128, S], f16, tag="kt", bufs=NKV, name=f"kt{i}")
               for i in range(NKV)]
        vts = [main.tile([128, NKV * HD], f16, tag="v", bufs=16, name=f"v{i}")
               for i in range(16)]
        # o_proj operands: per head-pair, [128 vdim, 2 head, 512 tok] fp8
        o8p = [main.tile([128, 2, SQ], fp8, tag="o8p", bufs=8, name=f"o8p{i}")
               for i in range(8)]
        or8p = [main.tile([128, 2, SQ], fp8, tag="or8p", bufs=8,
                          name=f"or8p{i}") for i in range(8)]
        ones64 = main.tile([128, 128], f16, tag="ones64", bufs=1)
        nc.vector.memset(ones64[:], WS)
        # h token-quarter 0 (this core's q rows) kept resident for Q proj
        h8q0 = main.tile([128, 8, 2, 512], fp8, tag="h8q0", bufs=1)
        hr8q0 = main.tile([128, 8, 2, 512], fp8, tag="hr8q0", bufs=1)

        def rope(dst, ps, cols, pool, evac="dve"):
            # x = [xr; xi] on partition halves, cc = [c; c], ss = [s; s].
            # Evacuate PSUM to fp16 once, then 4 fp16 ops at DVE 2x:
            #   A = x*cc, B = x*ss
            #   dst[0:64]   = A[0:64]   - B[64:128]
            #   dst[64:128] = A[64:128] + B[0:64]
            w = dst.shape[-1]
            xb = pool.tile([128, w], f16, tag="ropex", bufs=2, name="xb")
            ta = pool.tile([128, w], f16, tag="ropeA", bufs=2, name="ta")
            tb = pool.tile([128, w], f16, tag="ropeB", bufs=2, name="tb")
            with nc.allow_low_precision(reason="rope fp16 (64x scale)"):
                if evac == "act":
                    nc.scalar.copy(xb[:], ps[:])
                else:
                    nc.vector.tensor_copy(xb[:], ps[:])
                nc.vector.tensor_tensor(ta[:], xb[:], cc[:, cols], op=ALU.mult)
                # B = x*ss written partition-swapped so the combine ops read
                # equal base partitions (HW constraint for SB+SB operands)
                nc.vector.tensor_tensor(tb[0:64, :], xb[64:128, :],
                                        ss[64:128, cols], op=ALU.mult)
                nc.vector.tensor_tensor(tb[64:128, :], xb[0:64, :],
                                        ss[0:64, cols], op=ALU.mult)
                nc.vector.tensor_sub(dst[0:64, :], ta[0:64, :], tb[0:64, :])
                nc.vector.tensor_add(dst[64:128, :], ta[64:128, :],
                                     tb[64:128, :])

        def kv_proj(qt_, ha, hr, wk8, wkr8, wv8, wvr8):
            cols = bass.ts(qt_, 512)
            for g in range(NKV):
                ps = pp.tile([128, 512], f32, tag="mm", bufs=2, name="psk")
                for ti in range(8):
                    nc.tensor.matmul(ps[:], wk8[:, g, ti, :, :],
                                     ha[:, ti, :, :], start=(ti == 0),
                                     stop=False, perf_mode=DR)
                for ti in range(8):
                    nc.tensor.matmul(ps[:], wkr8[:, g, ti, :, :],
                                     ha[:, ti, :, :], start=False,
                                     stop=False, perf_mode=DR)
                for ti in range(8):
                    nc.tensor.matmul(ps[:], wk8[:, g, ti, :, :],
                                     hr[:, ti, :, :], start=False,
                                     stop=(ti == 7), perf_mode=DR)
                rope(kts[g][:, cols], ps, cols, main)
            for blk in range(4):
                ps = pp.tile([128, 512], f32, tag="mm", bufs=2, name="psv")
                bc = bass.ts(blk, 128)
                for ti in range(8):
                    nc.tensor.matmul(ps[:], ha[:, ti, :, bc],
                                     wv8[:, ti, :, :], start=(ti == 0),
                                     stop=False, perf_mode=DR)
                for ti in range(8):
                    nc.tensor.matmul(ps[:], ha[:, ti, :, bc],
                                     wvr8[:, ti, :, :], start=False,
                                     stop=False, perf_mode=DR)
                for ti in range(8):
                    nc.tensor.matmul(ps[:], hr[:, ti, :, bc],
                                     wv8[:, ti, :, :], start=False,
                                     stop=(ti == 7), perf_mode=DR)
                with nc.allow_low_precision(reason="v fp16 (64x scale)"):
                    nc.vector.tensor_copy(vts[qt_ * 4 + blk][:], ps[:])

        # ---------------- Q proj + attention pools -------------------------
        with tc.tile_pool(name="attn", bufs=1) as at:
            def qproj(m):
                wq8 = at.tile([128, 8, 2, 128], fp8, tag="wq8", bufs=3,
                              name="wq8")
                wqr8 = at.tile([128, 8, 2, 128], fp8, tag="wqr8", bufs=3,
                               name="wqr8")
                rows = bass.ts(m, 128)
                nc.sync.dma_start(
                    wq8[:], wq8D[rows, :].rearrange("p (t i c) -> p t i c",
                                                    t=8, i=2))
                nc.sync.dma_start(
                    wqr8[:], wqr8D[rows, :].rearrange("p (t i c) -> p t i c",
                                                      t=8, i=2))
                ps = pp.tile([128, 512], f32, tag="mm", bufs=2, name="psq")
                for ti in range(8):
                    nc.tensor.matmul(ps[:], wq8[:, ti, :, :],
                                     h8q0[:, ti, :, :], start=(ti == 0),
                                     stop=False, perf_mode=DR)
                for ti in range(8):
                    nc.tensor.matmul(ps[:], wqr8[:, ti, :, :],
                                     h8q0[:, ti, :, :], start=False,
                                     stop=False, perf_mode=DR)
                for ti in range(8):
                    nc.tensor.matmul(ps[:], wq8[:, ti, :, :],
                                     hr8q0[:, ti, :, :], start=False,
                                     stop=(ti == 7), perf_mode=DR)
                rope(qts[m], ps, slice(0, SQ), at, evac="act")

            # ---------------- K/V projections (split-fp8 DoubleRow) --------
            with tc.tile_pool(name="proj", bufs=1) as pt:
                nc.sync.dma_start(h8q0[:], h8D[:, 0, :, :, :])
                wk8 = pt.tile([128, NKV, 8, 2, 128], fp8, tag="wk8", bufs=1)
                wkr8 = pt.tile([128, NKV, 8, 2, 128], fp8, tag="wkr8", bufs=1)
                nc.sync.dma_start(
                    wk8[:], wk8D.rearrange("(g p) (t i c) -> p g t i c",
                                           p=128, t=8, i=2))
                nc.sync.dma_start(hr8q0[:], hr8D[:, 0, :, :, :])
                nc.sync.dma_start(
                    wkr8[:], wkr8D.rearrange("(g p) (t i c) -> p g t i c",
                                             p=128, t=8, i=2))
                nc.sync.dma_start(cc[:], ccD[:])
                nc.sync.dma_start(ss[:], ssD[:])
                wv8 = pt.tile([128, 8, 2, 512], fp8, tag="wv8", bufs=1)
                wvr8 = pt.tile([128, 8, 2, 512], fp8, tag="wvr8", bufs=1)
                nc.sync.dma_start(
                    wv8[:], wv8D.rearrange("p (t i n) -> p t i n", t=8, i=2))
                nc.sync.dma_start(
                    wvr8[:], wvr8D.rearrange("p (t i n) -> p t i n",
                                             t=8, i=2))

                kv_proj(0, h8q0, hr8q0, wk8, wkr8, wv8, wvr8)
                for qt_ in (1, 2, 3):
                    if qt_ == 3:
                        qproj(0)
                        qproj(1)
                    ha = pt.tile([128, 8, 2, 512], fp8, tag="h8", bufs=2,
                                 name="h8")
                    hr = pt.tile([128, 8, 2, 512], fp8, tag="hr8", bufs=2,
                                 name="hr8")
                    nc.sync.dma_start(ha[:], h8D[:, qt_, :, :, :])
                    nc.sync.dma_start(hr[:], hr8D[:, qt_, :, :, :])
                    kv_proj(qt_, ha, hr, wk8, wkr8, wv8, wvr8)

            # software pipeline: q projection for head h+2 is emitted before
            # head h's tail so rope(h+2) precedes head-h tail work in the DVE
            # stream (otherwise the PE stalls on qts[h+1] each head).
            for h in range(NH):
                g = h // (NH // NKV)
                pv = pp.tile([128, 512], f32, tag="pv", bufs=2, name="pv")
                # streamed pairwise tree for the rowsum: log-depth fp16
                # rounding (serial accumulation loses ~0.8% on denominators)
                tree = {}

                def tree_push(lvl, tile_):
                    if lvl in tree:
                        prev = tree.pop(lvl)
                        dst = at.tile([128, 1024], f16, tag=f"trs{lvl}",
                                      bufs=2, name=f"trs{lvl}")
                        nc.vector.tensor_add(dst[:], prev[:], tile_[:])
                        tree_push(lvl + 1, dst)
                    else:
                        tree[lvl] = tile_

                for jp in range(8):
                    sc = pp.tile([128, 1024], f32, tag="sc", bufs=2, name="sc")
                    for half in range(2):
                        j = 2 * jp + half
                        nc.tensor.matmul(
                            sc[:, bass.ts(half, 512)],
                            kts[g][:, bass.ts(j, 128)], qts[h][:],
                            start=True, stop=True,
                        )
                    ex = at.tile([128, 1024], f16, tag="ex", bufs=4,
                                 name=f"ex{jp}")
                    with nc.allow_low_precision(reason="probs fp16"):
                        nc.scalar.activation(ex[:], sc[:], FP.Exp,
                                             scale=SCALE / (WS * WS))
                    for half in range(2):
                        j = 2 * jp + half
                        nc.tensor.matmul(
                            pv[:], vts[j][:, bass.ts(g, 128)],
                            ex[:, bass.ts(half, 512)],
                            start=(jp == 0 and half == 0),
                            stop=(jp == 7 and half == 1),
                        )
                    # streamed pairwise rowsum accumulation (DVE 2x)
                    with nc.allow_low_precision(reason="rowsum fp16"):
                        tree_push(0, ex)
                if h + 2 < NH:
                    qproj(h + 2)
                with nc.allow_low_precision(reason="rowsum fp16"):
                    ex_acc = tree.pop(3)
                    assert not tree
                    fold = at.tile([128, 512], f16, tag="fold", bufs=2,
                                   name="fold")
                    nc.vector.tensor_add(fold[:], ex_acc[:, 0:512],
                                         ex_acc[:, 512:1024])
                rsb = pp.tile([128, 512], f32, tag="mm", bufs=2, name="rsb")
                nc.tensor.matmul(rsb[:], ones64[:], fold[:], start=True,
                                 stop=True)
                recipb = at.tile([128, 512], f32, tag="recipb", bufs=2,
                                 name="rc")
                with nc.allow_low_precision(reason="1/rowsum"):
                    nc.vector.reciprocal(recipb[:], rsb[:])
                    onorm = at.tile([128, 512], f16, tag="onorm", bufs=3,
                                    name="onorm")
                    nc.vector.tensor_tensor(onorm[:], pv[:], recipb[:],
                                            op=ALU.mult)
                    # fp8 split for o_proj (SBUF-only ops -> Pool engine)
                    tp, par = h // 2, h % 2
                    nc.gpsimd.tensor_copy(o8p[tp][:, par, :], onorm[:])
                    nc.gpsimd.tensor_tensor(or8p[tp][:, par, :], onorm[:],
                                            o8p[tp][:, par, :],
                                            op=ALU.subtract)

            # ---------------- output projection (split-fp8 DR) -------------
            with tc.tile_pool(name="oproj", bufs=1) as ot:
                for nq in range(4):
                    wo8 = ot.tile([128, 8, 2, 512], fp8, tag="wo8", bufs=2,
                                  name="wo8")
                    wor8 = ot.tile([128, 8, 2, 512], fp8, tag="wor8", bufs=2,
                                   name="wor8")
                    rows = bass.ts(nq, 128)
                    nc.sync.dma_start(
                        wo8[:], wo8D[rows, :].rearrange(
                            "p (t i n) -> p t i n", t=8, i=2))
                    nc.sync.dma_start(
                        wor8[:], wor8D[rows, :].rearrange(
                            "p (t i n) -> p t i n", t=8, i=2))
                    for blk in range(4):
                        pso = pp.tile([128, 512], f32, tag="mm", bufs=2,
                                      name="pso")
                        bc = bass.ts(blk, 128)
                        for ti in range(8):
                            nc.tensor.matmul(pso[:], o8p[ti][:, :, bc],
                                             wo8[:, ti, :, :],
                                             start=(ti == 0), stop=False,
                                             perf_mode=DR)
                        for ti in range(8):
                            nc.tensor.matmul(pso[:], o8p[ti][:, :, bc],
                                             wor8[:, ti, :, :], start=False,
                                             stop=False, perf_mode=DR)
                        for ti in range(8):
                            nc.tensor.matmul(pso[:], or8p[ti][:, :, bc],
                                             wo8[:, ti, :, :], start=False,
                                             stop=(ti == 7), perf_mode=DR)
                        o_s = ot.tile([128, 512], f32, tag="osb", bufs=2,
                                      name="osb")
                        nc.vector.tensor_copy(o_s[:], pso[:])
                        nc.sync.dma_start(
                            outD[bass.ts(blk, 128), bass.ts(nq, 512)], o_s[:])


def build(reps=1):
    nc = bacc.Bacc("TRN2", target_bir_lowering=False, debug=False,
                   num_devices=NC)
    t = {
        "h8": nc.dram_tensor("h8", [128, 4, 8, 2, 512], fp8,
                             kind="ExternalInput").ap(),
        "hr8": nc.dram_tensor("hr8", [128, 4, 8, 2, 512], fp8,
                              kind="ExternalInput").ap(),
        "wq8": nc.dram_tensor("wq8", [H, H], fp8, kind="ExternalInput").ap(),
        "wqr8": nc.dram_tensor("wqr8", [H, H], fp8, kind="ExternalInput").ap(),
        "wk8": nc.dram_tensor("wk8", [512, H], fp8, kind="ExternalInput").ap(),
        "wkr8": nc.dram_tensor("wkr8", [512, H], fp8,
                               kind="ExternalInput").ap(),
        "wv8": nc.dram_tensor("wv8", [128, 8192], fp8,
                              kind="ExternalInput").ap(),
        "wvr8": nc.dram_tensor("wvr8", [128, 8192], fp8,
                               kind="ExternalInput").ap(),
        "wo8": nc.dram_tensor("wo8", [512, 8192], fp8,
                              kind="ExternalInput").ap(),
        "wor8": nc.dram_tensor("wor8", [512, 8192], fp8,
                               kind="ExternalInput").ap(),
        "cc": nc.dram_tensor("cc", [128, S], f16, kind="ExternalInput").ap(),
        "ss": nc.dram_tensor("ss", [128, S], f16, kind="ExternalInput").ap(),
        "out": nc.dram_tensor("out", [SQ, H], f32, kind="ExternalOutput").ap(),
    }
    with tile.TileContext(nc) as tc:
        for _ in range(reps):
            _body(nc, tc, t)
    nc.compile()
    return nc


_ROPE_PERM = np.concatenate(
    [h * HD + np.r_[np.arange(0, HD, 2), np.arange(1, HD, 2)]
     for h in range(NH)]
)
_ROPE_PERM_KV = _ROPE_PERM[: NKV * HD]


def _split8(x):
    a = np.asarray(x, np8)
    r = np.asarray(x - a.astype(np.float32), np8)
    return a.astype(np.float32), r.astype(np.float32)


def _pack_h(hTp):
    # [2048 contr, 2048 tok] -> [128 p, 4 qt, 8 t, 2 i, 512 c]
    a = hTp.reshape(8, 2, 128, 4, 512)
    return np.ascontiguousarray(a.transpose(2, 3, 0, 1, 4)).astype(np8)


def _pack_w_stat(W, nblk):
    # stationary weights [2048 contr, nblk*128 out] ->
    # [nblk*128 rows=(m,p), 2048 cols=(t,i,c)]
    a = W.reshape(8, 2, 128, nblk, 128)
    return np.ascontiguousarray(
        a.transpose(3, 2, 0, 1, 4).reshape(nblk * 128, 2048)).astype(np8)


def _pack_w_mov(W, nq, n):
    # moving weights [2048 contr=(t,i,p), nq*n out] ->
    # [nq*128 rows=(q,p), 8*2*n cols=(t,i,n)]
    a = W.reshape(8, 2, 128, nq, n)
    return np.ascontiguousarray(
        a.transpose(3, 2, 0, 1, 4).reshape(nq * 128, 8 * 2 * n)).astype(np8)


def prep_inputs(hidden_states, freqs_cos, freqs_sin, Wq, Wk, Wv, Wo):
    """Host-side layout prep -> list of 8 per-core input maps."""
    wqT = np.ascontiguousarray(Wq.T[:, _ROPE_PERM]) * WS
    wkT = np.ascontiguousarray(Wk.T[:, _ROPE_PERM_KV]) * WS
    wvT = np.ascontiguousarray(Wv.T) * WS
    woT = np.ascontiguousarray(Wo.T) * WS

    wq8, wqr8 = _split8(wqT)
    wk8, wkr8 = _split8(wkT)
    wv8, wvr8 = _split8(wvT)
    wo8, wor8 = _split8(woT)

    weights = {
        "wq8": _pack_w_stat(wq8, 16), "wqr8": _pack_w_stat(wqr8, 16),
        "wk8": _pack_w_stat(wk8, 4), "wkr8": _pack_w_stat(wkr8, 4),
        "wv8": _pack_w_mov(wv8, 1, 512), "wvr8": _pack_w_mov(wvr8, 1, 512),
        "wo8": _pack_w_mov(wo8, 4, 512), "wor8": _pack_w_mov(wor8, 4, 512),
    }

    cosT = freqs_cos.T  # [64, S]
    sinT = freqs_sin.T
    ccN = np.concatenate([cosT, cosT], 0)  # [128, S]
    ssN = np.concatenate([sinT, sinT], 0)
    in_maps = []
    for c in range(NC):
        b, chunk = divmod(c, 4)
        sq0 = chunk * SQ
        perm = np.r_[sq0: sq0 + SQ, 0:sq0, sq0 + SQ: S]
        hTc = np.ascontiguousarray(hidden_states[b].T[:, perm])
        a8, r8 = _split8(hTc)
        in_maps.append({
            "h8": _pack_h(a8), "hr8": _pack_h(r8),
            "cc": np.ascontiguousarray(ccN[:, perm]).astype(np.float16),
            "ss": np.ascontiguousarray(ssN[:, perm]).astype(np.float16),
            **weights,
        })
    return in_maps


_CACHE = {}


def _get_nc(reps=1):
    if reps not in _CACHE:
        _CACHE[reps] = build(reps)
    return _CACHE[reps]


def kernel(hidden_states, freqs_cos, freqs_sin, Wq, Wk, Wv, Wo):
    in_maps = prep_inputs(
        np.asarray(hidden_states, np.float32),
        np.asarray(freqs_cos, np.float32),
        np.asarray(freqs_sin, np.float32),
        np.asarray(Wq, np.float32),
        np.asarray(Wk, np.float32),
        np.asarray(Wv, np.float32),
        np.asarray(Wo, np.float32),
    )
    nc = _get_nc(int(os.environ.get("KERNEL_REPS", "1")))
    res = run_bass_kernel_spmd(nc, in_maps, core_ids=list(range(NC)))
    out = np.empty((B, S, H), np.float32)
    for c in range(NC):
        b, chunk = divmod(c, 4)
        out[b, chunk * SQ: (chunk + 1) * SQ, :] = res.results[c]["out"] * (
            1.0 / WS)
    return out
